# revision 17
# baseline (speedup 1.0000x reference)
"""Trainium2 Bass kernel for top-2-of-8 MoE routing (nn_MoETopX).

Reference semantics (computed densely there, routed here):
    gate_logits = x @ Wg + bg                       # [N, 8]
    top_vals, top_idx = top_k(gate_logits, 2)
    w = softmax(softmax(top_vals))                  # double softmax, [N, 2]
    h_e = x @ We[e] + be[e]       for the 2 selected experts per token
    y_e = softmax(relu(h_e), axis=-1)
    out = sum_e w_e * y_e                           # [N, 2048]

Strategy: data-parallel over tokens on 8 NeuronCores, no collectives.
Each core owns NTOK tokens and locally:
  1. Routed expert matmuls in bf16 over host-gathered token slots (tokens
     duplicated per selected expert, grouped into NSLOT weight slots; the
     slot->expert binding is pure host DATA -- the host packs each core's
     weight tensor -- so one compiled program serves any assignment).
     Tokens are assigned to cores by a small transportation LP so each
     core only touches 5 of the 8 experts (T=17 slot tiles, 40MB of
     weight traffic); falls back to an 8-slot layout when the LP or
     scipy is unavailable.
  2. The double-softmax combine weight per slot row is host data (wsl);
     the host already computes the full gate for routing, so the device
     spends no PE/DVE time on it.
  3. bias+relu+exp via exp(relu(h+be)) == exp(max(h,-be))*exp(be): the
     bias rides DVE max/mult ops against host-prebroadcast [128,O] rows
     (plain DMA); row-sum via tensor_reduce; rows scaled by w/sum(exp)
     and cast to bf16.
  4. Each scaled row tile is indirect-scatter-DMA'd (write-only bypass,
     no RMW) into a token-ordered DRAM buffer: rank0 rows to [0,NTOK),
     rank1 rows to [NTOK,2*NTOK) -- all destinations distinct, padding
     lanes land in junk rows. Core-local token ids are ordered by the
     last slot tile that feeds them, so each 128-token output m-tile's
     combine (two contiguous HWDGE reads + DVE add + output write, all
     on the scalar queue) fires right after the scatter of its last
     tile and overlaps later tiles' matmuls instead of serializing
     into a tail.

Host python does the integer routing metadata (slot lists, capacities,
scatter indices), the gate-weight sigmoids, and layout/dtype prep; the
expert FLOPs (99.7% of reference FLOPs) run on device.
"""

import numpy as np
import ml_dtypes

import concourse.bass as bass
import concourse.tile as tile
from concourse import bacc, mybir
from concourse.bass_utils import run_bass_kernel_spmd

F32 = mybir.dt.float32
BF16 = mybir.dt.bfloat16
I32 = mybir.dt.int32

N_CORES = 8
N_TOKENS = 8192
NTOK = N_TOKENS // N_CORES  # 1024 tokens per core
MT = NTOK // 128            # 8 output m-tiles per core
D = 2048
O = 2048
E = 8
KC = D // 128   # 16 contraction chunks
OH = 4          # output-dim quarters (matmul out must fit one PSUM bank)
OHW = O // OH   # 512
GCH = 512       # tokens per xg chunk (4 tiles)
NWARM = 72      # PE warm-up matmuls issued during the initial DMA wait

# Expert-cluster designs (randomized-search + LP on the reference data
# distribution): blocks[c] = 5 experts of core c, bigs[c] = its "big"
# experts (4-tile slots, cap 512 routed slots; "small" = 3-tile, cap 384).
# Tried in order; first feasible wins. T=17 design first (tighter), then
# the roomier T=18 design.
CLUSTER_DESIGNS = [
    # T=17: 2 big + 3 small per core, slot_caps (4,4,3,3,3), margin 4
    dict(blocks=[(2, 3, 4, 6, 7), (0, 2, 3, 5, 6), (2, 4, 5, 6, 7),
                 (0, 1, 5, 6, 7), (1, 4, 5, 6, 7), (0, 1, 3, 4, 7),
                 (0, 1, 2, 3, 4), (0, 1, 2, 3, 5)],
         bigs=[(3, 7), (0, 3), (2, 5), (1, 6), (4, 6), (0, 7),
               (1, 4), (2, 5)],
         margin=4),
    # T=18: 3 big + 2 small per core, slot_caps (4,4,4,3,3), margin 8
    dict(blocks=[(2, 3, 4, 5, 6), (0, 1, 3, 4, 5), (2, 3, 4, 6, 7),
                 (0, 1, 3, 5, 7), (0, 1, 2, 4, 7), (1, 4, 5, 6, 7),
                 (0, 2, 3, 5, 6), (0, 1, 2, 6, 7)],
         bigs=[(2, 3, 4), (1, 3, 4), (3, 6, 7), (0, 5, 7),
               (0, 4, 7), (1, 5, 6), (2, 5, 6), (0, 1, 2)],
         margin=8),
]
BIG_CAP, SMALL_CAP = 512, 384


# ----------------------------------------------------------------------------
# Host-side routing metadata
# ----------------------------------------------------------------------------

def _host_route(x, Wg, bg):
    """fp32 gate; top-2 per token (matches jax.lax.top_k tie order)."""
    logits = (x.astype(np.float32) @ Wg.astype(np.float32)) + bg.astype(np.float32)
    order = np.argsort(-logits, axis=1, kind="stable")
    return logits, order[:, :2].astype(np.int32)


def _cluster_assign(top2):
    """Token->core assignment where each core touches only 5 experts.
    Returns (slot_caps, slot_experts_per_core, cores) or None."""
    try:
        from scipy.optimize import linprog
    except ImportError:
        return None
    for design in CLUSTER_DESIGNS:
        r = _cluster_assign_one(top2, linprog, design["blocks"],
                                [frozenset(b) for b in design["bigs"]],
                                design["margin"])
        if r is not None:
            return r
    return None


def _cluster_assign_one(top2, linprog, blocks, bigs, margin):
    pairs = [(a, b) for a in range(E) for b in range(a + 1, E)]
    pr = np.sort(top2, axis=1)
    pid = pr[:, 0] * E + pr[:, 1]
    n = {p: int(np.sum(pid == p[0] * E + p[1])) for p in pairs}

    if any(n[p] > 0 and not any(set(p) <= set(blocks[c])
                                for c in range(N_CORES)) for p in pairs):
        return None
    var = [(p, c) for p in pairs for c in range(N_CORES)
           if set(p) <= set(blocks[c])]
    vi = {v: i for i, v in enumerate(var)}
    nv = len(var)
    A_eq, b_eq = [], []
    for p in pairs:
        if n[p] == 0:
            continue
        row = np.zeros(nv)
        for c in range(N_CORES):
            if (p, c) in vi:
                row[vi[(p, c)]] = 1
        A_eq.append(row)
        b_eq.append(n[p])
    A_ub, b_ub = [], []
    for c in range(N_CORES):
        row = np.zeros(nv)
        for p in pairs:
            if (p, c) in vi:
                row[vi[(p, c)]] = 1
        A_ub.append(row)
        b_ub.append(NTOK)
        for e in blocks[c]:
            row = np.zeros(nv)
            for p in pairs:
                if e in p and (p, c) in vi:
                    row[vi[(p, c)]] = 1
            A_ub.append(row)
            b_ub.append((BIG_CAP if e in bigs[c] else SMALL_CAP) - margin)
    res = linprog(np.zeros(nv), A_ub=np.array(A_ub), b_ub=np.array(b_ub),
                  A_eq=np.array(A_eq), b_eq=np.array(b_eq),
                  bounds=[(0, None)] * nv, method='highs')
    if res.status != 0:
        return None
    x = res.x

    cores = np.full(top2.shape[0], -1, dtype=int)
    ecount = np.zeros((N_CORES, E), int)
    tcount = np.zeros(N_CORES, int)
    for p in pairs:
        if n[p] == 0:
            continue
        toks = np.where(pid == p[0] * E + p[1])[0]
        elig = [c for c in range(N_CORES) if (p, c) in vi]
        vals = np.array([x[vi[(p, c)]] for c in elig])
        ints = np.floor(vals).astype(int)
        rem = n[p] - ints.sum()
        frac = vals - ints
        for idx in np.argsort(-frac)[:rem]:
            ints[idx] += 1
        off = 0
        for c, k in zip(elig, ints.tolist()):
            cores[toks[off:off + k]] = c
            ecount[c, p[0]] += k
            ecount[c, p[1]] += k
            tcount[c] += k
            off += k
    for c in range(N_CORES):
        if tcount[c] > NTOK:
            return None
        for e in range(E):
            if e in blocks[c]:
                cap = BIG_CAP if e in bigs[c] else SMALL_CAP
                if ecount[c, e] > cap:
                    return None
            elif ecount[c, e] > 0:
                return None
    nbig = len(bigs[0])
    slot_caps = (4,) * nbig + (3,) * (5 - nbig)
    slot_experts = [sorted(bigs[c]) + sorted(set(blocks[c]) - bigs[c])
                    for c in range(N_CORES)]
    return slot_caps, slot_experts, cores


def _balance_tokens(top2):
    """Fallback: every core gets all 8 experts with shared per-expert caps."""
    g = np.bincount(top2.reshape(-1), minlength=E)
    cap_tiles = np.maximum(1, np.ceil(g / (128 * N_CORES)).astype(int))
    for _attempt in range(8):
        cap = cap_tiles * 128
        rem = np.tile(cap, (N_CORES, 1)).astype(int)
        ntok = np.zeros(N_CORES, dtype=int)
        cores = np.full(N_TOKENS, -1, dtype=int)
        slack = N_CORES * cap - g
        tok_score = np.minimum(slack[top2[:, 0]], slack[top2[:, 1]])
        order = np.argsort(tok_score, kind="stable")
        failed_expert = -1
        for t in order:
            e1, e2 = top2[t]
            room = np.minimum(rem[:, e1], rem[:, e2]).astype(float)
            room[ntok >= NTOK] = -1
            c = int(np.argmax(room + 1e-3 * rem.sum(axis=1)))
            if room[c] <= 0:
                failed_expert = e1 if rem[:, e1].max() <= 0 else e2
                break
            cores[t] = c
            rem[c, e1] -= 1
            rem[c, e2] -= 1
            ntok[c] += 1
        else:
            return tuple(int(c) for c in cap_tiles), cores
        cap_tiles[failed_expert] += 1
    raise RuntimeError("token balancing failed")


def _sigmoid(v):
    return 1.0 / (1.0 + np.exp(-v))


def _default_sched(T):
    """Combine m-tile m after `sched[m]` slot tiles have completed."""
    return tuple(min(T - (MT - 1) + m, T) for m in range(MT))


def _earliest_sched(T, maxtiles):
    """Tightest nondecreasing schedule: combine m fires once 128*(m+1)
    tokens (by sorted max slot tile) are complete, on every core."""
    sched = []
    prev = 1
    for m in range(MT):
        q = prev
        for mt in maxtiles:
            q = max(q, int(mt[128 * (m + 1) - 1]) + 1)
        sched.append(min(q, T))
        prev = sched[-1]
    sched[-1] = T
    return tuple(sched)


def _prepare_core(x, logits, top2, tok_ids, slot_experts, slot_caps):
    """Build one core's host arrays: gathered activations (xg), scatter
    indices (sc) and double-softmax slot weights (wsl). Returns
    (part, ordered_tok_ids, sorted_maxtile)."""
    nreal = len(tok_ids)
    assert nreal == NTOK, nreal
    t2 = top2[tok_ids]                              # [NTOK, 2]
    T = int(sum(slot_caps))
    S = T * 128
    NCH = (T + 3) // 4
    SP = NCH * GCH

    slot_tok = np.full(S, -1, dtype=np.int64)       # core-local token idx
    dvals = np.zeros(S, dtype=np.float32)           # v_self - v_other
    rank1 = np.zeros(S, dtype=bool)
    tile_of = np.zeros((NTOK, 2), dtype=int)        # [token, rank] -> tile
    off = 0
    for j, e in enumerate(slot_experts):
        sel = np.where((t2[:, 0] == e) | (t2[:, 1] == e))[0]
        assert len(sel) <= slot_caps[j] * 128, (j, e, len(sel))
        n = len(sel)
        sl = slice(off, off + n)
        slot_tok[sl] = sel
        other = np.where(t2[sel, 0] == e, t2[sel, 1], t2[sel, 0])
        gt = logits[tok_ids[sel]]
        dvals[sl] = gt[np.arange(n), e] - gt[np.arange(n), other]
        first = t2[sel, 0] == e
        rank1[off + np.arange(n)] = ~first
        tiles = off // 128 + np.arange(n) // 128
        tile_of[sel, np.where(first, 0, 1)] = tiles
        off += slot_caps[j] * 128

    # order core-local token ids by the last tile that feeds them, so
    # m-tile m's combine can fire as soon as its tiles are scattered
    maxtile = tile_of.max(axis=1)
    order = np.argsort(maxtile, kind="stable")
    newid = np.empty(NTOK, dtype=np.int64)
    newid[order] = np.arange(NTOK)

    real = slot_tok >= 0
    # double softmax over the top-2 pair, seen from this slot's expert
    wv = np.zeros(S, dtype=np.float32)
    wv[real] = _sigmoid(2.0 * _sigmoid(dvals[real]) - 1.0)
    lanes = np.arange(S) % 128
    sc = np.where(real,
                  2 * newid[np.where(real, slot_tok, 0)] + rank1,
                  2 * NTOK + lanes).astype(np.int32)

    # gathered slot activations, chunk-major & zero-padded:
    # XG[c, p, k, i] = x[tok(slot 512c+i), 128k+p]
    xs = np.zeros((SP, D), dtype=np.float32)
    xs[:S][real] = x[tok_ids[slot_tok[real]]]
    XG = np.ascontiguousarray(
        xs.reshape(NCH, GCH, KC, 128).transpose(0, 3, 2, 1)
    ).astype(ml_dtypes.bfloat16)

    part = {
        "xg": XG,                                                  # [NCH,128,KC,GCH]
        "sc": np.ascontiguousarray(
            sc.reshape(T, 128).T.astype(np.int32)),                # [128, T]
        "wsl": np.ascontiguousarray(
            wv.reshape(T, 128).T.astype(np.float32)),              # [128, T]
    }
    return part, tok_ids[order], maxtile[order]


def _pack_weights(We, be, slot_experts):
    idx = np.asarray(slot_experts, dtype=np.int64)
    NSLOT = len(idx)
    WSEG = np.ascontiguousarray(
        We[idx].reshape(NSLOT, KC, 128, OH, OHW).transpose(0, 3, 2, 1, 4)
    ).astype(ml_dtypes.bfloat16)
    bsel = be[idx].astype(np.float32)
    rows = np.stack([-bsel, np.exp(bsel)], axis=1).astype(ml_dtypes.bfloat16)
    BSEG = np.ascontiguousarray(
        np.broadcast_to(rows[:, :, None, :], (NSLOT, 2, 128, O)))
    return WSEG, BSEG


# ----------------------------------------------------------------------------
# Device program
# ----------------------------------------------------------------------------

def build_program(slot_caps, sched):
    slot_caps = tuple(int(c) for c in slot_caps)
    NSLOT = len(slot_caps)
    T = sum(slot_caps)
    NCH = (T + 3) // 4

    nc = bacc.Bacc("TRN2", target_bir_lowering=False, debug=False,
                   num_devices=N_CORES)

    xg = nc.dram_tensor("xg", [NCH, 128, KC, GCH], BF16,
                        kind="ExternalInput").ap()
    wseg = nc.dram_tensor("wseg", [NSLOT, OH, 128, KC, OHW], BF16,
                          kind="ExternalInput").ap()
    bseg = nc.dram_tensor("bseg", [NSLOT, 2, 128, O], BF16,
                          kind="ExternalInput").ap()
    scd = nc.dram_tensor("sc", [128, T], I32, kind="ExternalInput").ap()
    wsld = nc.dram_tensor("wsl", [128, T], F32, kind="ExternalInput").ap()
    out = nc.dram_tensor("out", [NTOK, O], BF16, kind="ExternalOutput").ap()
    # token-major rows: token i's rank0/rank1 rows adjacent at flat rows
    # 2i/2i+1; junk pad rows after 2*NTOK. The combine reads one [128,2*O]
    # block per m-tile; the scatter writes through the flat [.,O] view.
    rowsd = nc.dram_tensor("rowsd", [NTOK + 64, 2 * O], BF16).ap()
    rowsflat = rowsd.rearrange("n (two o) -> (n two) o", two=2)

    AF = mybir.ActivationFunctionType
    ALU = mybir.AluOpType

    with tile.TileContext(nc) as tc:
        with (
            tc.tile_pool(name="singles", bufs=1) as singles,
            tc.tile_pool(name="xgp", bufs=3) as xgp,
            tc.tile_pool(name="wpool", bufs=6) as wpool,
            tc.tile_pool(name="mpsum", bufs=6, space="PSUM") as mpsum,
            tc.tile_pool(name="wps", bufs=1, space="PSUM") as wpsp,
            tc.tile_pool(name="berp", bufs=2) as berp,
            tc.tile_pool(name="rowp", bufs=max(slot_caps) + 2) as rowp,
            tc.tile_pool(name="smallp", bufs=4) as smallp,
            tc.tile_pool(name="combp", bufs=2) as combp,
        ):
            # ---- PE warm-up: dependency-free matmuls flip the HAM clock
            # gate to 8/8 during the initial DMA wait.
            wz = singles.tile([128, 64], BF16)
            nc.vector.memset(wz, 0.0)
            wps = wpsp.tile([64, 64], F32)
            for i in range(NWARM):
                nc.tensor.matmul(wps, lhsT=wz, rhs=wz,
                                 start=(i % 8 == 0), stop=(i % 8 == 7))

            # ---- small shared inputs + zero-fill of out (scalar queue)
            sc_sb = singles.tile([128, T], I32)
            nc.scalar.dma_start(out=sc_sb, in_=scd)
            wsl_sb = singles.tile([128, T], F32)
            nc.scalar.dma_start(out=wsl_sb, in_=wsld)

            # ---- xg chunks + weight chunks interleaved on ONE queue (sync)
            # in consumption order; the first chunk/weight are split so the
            # first matmuls start as early as possible.
            xgc = [None] * NCH
            wsb = {}
            _off = 0
            for j in range(NSLOT):
                _t0, _t1 = _off, _off + slot_caps[j]
                _off = _t1
                for c in range(_t0 // 4, (_t1 - 1) // 4 + 1):
                    if xgc[c] is None:
                        xt = xgp.tile([128, KC, GCH], BF16, tag="xgc",
                                      name=f"xgc{c}")
                        if c > 0:
                            nc.sync.dma_start(out=xt, in_=xg[c])
                        xgc[c] = xt
                wtiles = []
                for oh in range(OH):
                    w = wpool.tile([128, KC, OHW], BF16, tag="wsb",
                                   name=f"w{j}_{oh}")
                    wsb[(j, oh)] = w
                    wtiles.append((w, wseg[j, oh]))
                if j == 0:
                    # interleave quarter-loads of the first chunk/weight so
                    # the first matmul groups start as early as possible
                    xt = xgc[0]
                    for q in range(4):
                        ksl = slice(4 * q, 4 * q + 4)
                        nc.sync.dma_start(out=xt[:, ksl, :], in_=xg[0, :, ksl, :])
                        nc.sync.dma_start(out=wtiles[0][0][:, ksl, :],
                                          in_=wtiles[0][1][:, ksl, :])
                    for w, src in wtiles[1:]:
                        nc.sync.dma_start(out=w, in_=src)
                else:
                    for w, src in wtiles:
                        nc.sync.dma_start(out=w, in_=src)

            pending = {}
            # reads deferred one tile past sched; end-of-kernel combines are
            # held past the last scatter so they cannot WAR-block it
            rsched = [sched[m] + 1 if sched[m] < T - 2 else T + 1
                      for m in range(MT)]

            def combine_read(m):
                # token p's two rows land adjacent: one 1MB contiguous read
                ab = combp.tile([128, 2 * O], BF16, tag="ab")
                nc.scalar.dma_start(out=ab,
                                    in_=rowsd[m * 128:(m + 1) * 128, :])
                pending[m] = ab

            def combine_add(m):
                ab = pending.pop(m)
                nc.vector.tensor_tensor(out=ab[:, :O], in0=ab[:, :O],
                                        in1=ab[:, O:], op=ALU.add)
                nc.scalar.dma_start(out=out[m * 128:(m + 1) * 128, :],
                                    in_=ab[:, :O])

            # ---- main loop: slot-major, oh-pass inside (weight chunks are
            # short-lived). exp(relu(h+be)) == exp(max(h,-be)) * exp(be):
            # the bias rides DVE ops with prebroadcast rows.
            negbe = {}
            expbe = {}

            def emit_ber(j):
                if j >= NSLOT or j in negbe:
                    return
                nb = berp.tile([128, O], BF16, tag="nrow", name=f"nrow{j}")
                nc.scalar.dma_start(out=nb, in_=bseg[j][0])
                eb = berp.tile([128, O], BF16, tag="erow", name=f"erow{j}")
                nc.scalar.dma_start(out=eb, in_=bseg[j][1])
                negbe[j] = nb
                expbe[j] = eb

            emit_ber(0)
            emit_ber(1)
            tiles_done = 0
            tile_off = 0
            for j in range(NSLOT):
                t0, t1 = tile_off, tile_off + slot_caps[j]
                tile_off = t1
                emit_ber(j + 1)
                rowbufs = {}
                sums = {}
                if j == 0:
                    touring = [(oh, t) for oh in range(OH)
                               for t in range(t0, t1)]
                else:
                    touring = [(oh, t) for g in (0, 2)
                               for t in range(t0, t1) for oh in (g, g + 1)]
                for oh, t in touring:
                        if oh == 0:
                            rowbufs[t] = rowp.tile([128, O], BF16, tag="rowbuf",
                                                   name=f"rowbuf{t}")
                            sums[t] = smallp.tile([128, OH], F32, tag="sums",
                                                  name=f"sums{t}")
                        ps = mpsum.tile([128, OHW], F32)
                        for k in range(KC):
                            nc.tensor.matmul(
                                ps,
                                lhsT=xgc[t // 4][:, k, (t % 4) * 128:
                                                 (t % 4) * 128 + 128],
                                rhs=wsb[(j, oh)][:, k, :],
                                start=(k == 0), stop=(k == KC - 1))
                        ohsl = slice(oh * OHW, (oh + 1) * OHW)
                        seg = rowbufs[t][:, ohsl]
                        nc.vector.tensor_tensor(out=seg, in0=ps,
                                                in1=negbe[j][:, ohsl],
                                                op=ALU.max)
                        nc.scalar.activation(seg, seg, AF.Exp)
                        nc.vector.tensor_tensor(out=seg, in0=seg,
                                                in1=expbe[j][:, ohsl],
                                                op=ALU.mult)
                        nc.vector.tensor_reduce(sums[t][:, oh:oh + 1], seg,
                                                axis=mybir.AxisListType.X,
                                                op=ALU.add)
                        if oh < OH - 1:
                            continue
                        # w/sum(exp) scale, then scatter-accumulate the
                        # tile's rows straight into the output tokens.
                        stot = smallp.tile([128, 1], F32, tag="stot")
                        nc.vector.tensor_reduce(stot, sums[t],
                                                axis=mybir.AxisListType.X,
                                                op=ALU.add)
                        nc.vector.reciprocal(stot, stot)
                        scl = smallp.tile([128, 1], F32, tag="scl")
                        nc.vector.tensor_tensor(out=scl, in0=stot,
                                                in1=wsl_sb[:, t:t + 1],
                                                op=ALU.mult)
                        nc.vector.tensor_scalar_mul(rowbufs[t], rowbufs[t],
                                                    scl[:, :1])
                        nc.gpsimd.indirect_dma_start(
                            out=rowsflat,
                            out_offset=bass.IndirectOffsetOnAxis(
                                ap=sc_sb[:, t:t + 1], axis=0),
                            in_=rowbufs[t], in_offset=None)
                        del rowbufs[t], sums[t]
                        tiles_done += 1
                        for m in range(MT):
                            if rsched[m] == tiles_done - 1:
                                combine_add(m)
                        for m in range(MT):
                            if rsched[m] == tiles_done:
                                combine_read(m)
            for m in range(MT):
                if rsched[m] > T:
                    combine_read(m)
            for m in sorted(pending):
                combine_add(m)

    nc.compile()
    return nc


_PROGRAM_CACHE = {}


def _get_program(key):
    if key not in _PROGRAM_CACHE:
        slot_caps, sched = key
        _PROGRAM_CACHE[key] = build_program(slot_caps, sched)
    return _PROGRAM_CACHE[key]


def make_in_maps(inputs, We, be, Wg, bg):
    """Returns (program_key, core_token_ids, in_maps)."""
    x = np.asarray(inputs, dtype=np.float32)
    We = np.asarray(We, dtype=np.float32)
    be = np.asarray(be, dtype=np.float32)
    Wg = np.asarray(Wg, dtype=np.float32)
    bg = np.asarray(bg, dtype=np.float32)

    logits, top2 = _host_route(x, Wg, bg)
    clus = _cluster_assign(top2)
    if clus is not None:
        slot_caps, slot_experts, cores = clus
    else:
        slot_caps, cores = _balance_tokens(top2)
        slot_experts = [list(range(E))] * N_CORES

    T = sum(slot_caps)
    parts, core_tok, maxtiles = [], [], []
    for c in range(N_CORES):
        tok = np.where(cores == c)[0]
        part, tok_ordered, mt = _prepare_core(
            x, logits, top2, tok, slot_experts[c], slot_caps)
        parts.append((part, slot_experts[c]))
        core_tok.append(tok_ordered)
        maxtiles.append(mt)
    sched = _earliest_sched(T, maxtiles)

    in_maps = []
    for c in range(N_CORES):
        part, sexp = parts[c]
        WSEG, BSEG = _pack_weights(We, be, sexp)
        part["wseg"] = WSEG
        part["bseg"] = BSEG
        in_maps.append(part)
    return (tuple(slot_caps), sched), core_tok, in_maps


def kernel(inputs, We, be, Wg, bg, top_x):
    assert int(top_x) == 2, "kernel specialized for top_x=2"
    key, core_tok, in_maps = make_in_maps(inputs, We, be, Wg, bg)
    nc = _get_program(key)
    res = run_bass_kernel_spmd(nc, in_maps, list(range(N_CORES)))
    full = np.empty((N_TOKENS, O), dtype=np.float32)
    for c in range(N_CORES):
        full[core_tok[c]] = np.asarray(res.results[c]["out"],
                                       dtype=np.float32)[:NTOK]
    return full


# revision 19
# speedup vs baseline: 1.0710x; 1.0710x over previous
"""Trainium2 Bass kernel for top-2-of-8 MoE routing (nn_MoETopX).

Reference semantics (computed densely there, routed here):
    gate_logits = x @ Wg + bg                       # [N, 8]
    top_vals, top_idx = top_k(gate_logits, 2)
    w = softmax(softmax(top_vals))                  # double softmax, [N, 2]
    h_e = x @ We[e] + be[e]       for the 2 selected experts per token
    y_e = softmax(relu(h_e), axis=-1)
    out = sum_e w_e * y_e                           # [N, 2048]

Strategy: data-parallel over tokens on 8 NeuronCores, no collectives.
Each core owns NTOK tokens and locally:
  1. Routed expert matmuls in bf16 over host-gathered token slots (tokens
     duplicated per selected expert, grouped into NSLOT weight slots; the
     slot->expert binding is pure host DATA -- the host packs each core's
     weight tensor -- so one compiled program serves any assignment).
     Tokens are assigned to cores by a small transportation LP so each
     core only touches 5 of the 8 experts (T=17 slot tiles, 40MB of
     weight traffic); falls back to an 8-slot layout when the LP or
     scipy is unavailable.
  2. The double-softmax combine weight per slot row is host data (wsl);
     the host already computes the full gate for routing, so the device
     spends no PE/DVE time on it.
  3. bias+relu+exp via exp(relu(h+be)) == exp(max(h,-be))*exp(be): the
     bias rides DVE max/mult ops against host-prebroadcast [128,O] rows
     (plain DMA); row-sum via tensor_reduce; rows scaled by w/sum(exp)
     and cast to bf16.
  4. Each scaled row tile is indirect-scatter-DMA'd (write-only bypass,
     no RMW) into a token-ordered DRAM buffer: rank0 rows to [0,NTOK),
     rank1 rows to [NTOK,2*NTOK) -- all destinations distinct, padding
     lanes land in junk rows. Core-local token ids are ordered by the
     last slot tile that feeds them, so each 128-token output m-tile's
     combine (two contiguous HWDGE reads + DVE add + output write, all
     on the scalar queue) fires right after the scatter of its last
     tile and overlaps later tiles' matmuls instead of serializing
     into a tail.

Host python does the integer routing metadata (slot lists, capacities,
scatter indices), the gate-weight sigmoids, and layout/dtype prep; the
expert FLOPs (99.7% of reference FLOPs) run on device.
"""

import numpy as np
import ml_dtypes

import concourse.bass as bass
import concourse.tile as tile
from concourse import bacc, mybir
from concourse.bass_utils import run_bass_kernel_spmd

F32 = mybir.dt.float32
BF16 = mybir.dt.bfloat16
I32 = mybir.dt.int32

N_CORES = 8
N_TOKENS = 8192
NTOK = N_TOKENS // N_CORES  # 1024 tokens per core
MT = NTOK // 128            # 8 output m-tiles per core
D = 2048
O = 2048
E = 8
KC = D // 128   # 16 contraction chunks
OH = 4          # output-dim quarters (matmul out must fit one PSUM bank)
OHW = O // OH   # 512
GCH = 512       # tokens per xg chunk (4 tiles)
NWARM = 72      # PE warm-up matmuls issued during the initial DMA wait

# Expert-cluster designs (randomized-search + LP on the reference data
# distribution): blocks[c] = 5 experts of core c, bigs[c] = its "big"
# experts (4-tile slots, cap 512 routed slots; "small" = 3-tile, cap 384).
# Tried in order; first feasible wins. T=17 design first (tighter), then
# the roomier T=18 design.
CLUSTER_DESIGNS = [
    # T=17: 2 big + 3 small per core, slot_caps (4,4,3,3,3), margin 4
    dict(blocks=[(2, 3, 4, 6, 7), (0, 2, 3, 5, 6), (2, 4, 5, 6, 7),
                 (0, 1, 5, 6, 7), (1, 4, 5, 6, 7), (0, 1, 3, 4, 7),
                 (0, 1, 2, 3, 4), (0, 1, 2, 3, 5)],
         bigs=[(3, 7), (0, 3), (2, 5), (1, 6), (4, 6), (0, 7),
               (1, 4), (2, 5)],
         margin=4),
    # T=18: 3 big + 2 small per core, slot_caps (4,4,4,3,3), margin 8
    dict(blocks=[(2, 3, 4, 5, 6), (0, 1, 3, 4, 5), (2, 3, 4, 6, 7),
                 (0, 1, 3, 5, 7), (0, 1, 2, 4, 7), (1, 4, 5, 6, 7),
                 (0, 2, 3, 5, 6), (0, 1, 2, 6, 7)],
         bigs=[(2, 3, 4), (1, 3, 4), (3, 6, 7), (0, 5, 7),
               (0, 4, 7), (1, 5, 6), (2, 5, 6), (0, 1, 2)],
         margin=8),
]
BIG_CAP, SMALL_CAP = 512, 384


# ----------------------------------------------------------------------------
# Host-side routing metadata
# ----------------------------------------------------------------------------

def _host_route(x, Wg, bg):
    """fp32 gate; top-2 per token (matches jax.lax.top_k tie order)."""
    logits = (x.astype(np.float32) @ Wg.astype(np.float32)) + bg.astype(np.float32)
    order = np.argsort(-logits, axis=1, kind="stable")
    return logits, order[:, :2].astype(np.int32)


def _cluster_assign(top2):
    """Token->core assignment where each core touches only 5 experts.
    Returns (slot_caps, slot_experts_per_core, cores) or None."""
    try:
        from scipy.optimize import linprog
    except ImportError:
        return None
    for design in CLUSTER_DESIGNS:
        r = _cluster_assign_one(top2, linprog, design["blocks"],
                                [frozenset(b) for b in design["bigs"]],
                                design["margin"])
        if r is not None:
            return r
    return None


def _cluster_assign_one(top2, linprog, blocks, bigs, margin):
    pairs = [(a, b) for a in range(E) for b in range(a + 1, E)]
    pr = np.sort(top2, axis=1)
    pid = pr[:, 0] * E + pr[:, 1]
    n = {p: int(np.sum(pid == p[0] * E + p[1])) for p in pairs}

    if any(n[p] > 0 and not any(set(p) <= set(blocks[c])
                                for c in range(N_CORES)) for p in pairs):
        return None
    var = [(p, c) for p in pairs for c in range(N_CORES)
           if set(p) <= set(blocks[c])]
    vi = {v: i for i, v in enumerate(var)}
    nv = len(var)
    A_eq, b_eq = [], []
    for p in pairs:
        if n[p] == 0:
            continue
        row = np.zeros(nv)
        for c in range(N_CORES):
            if (p, c) in vi:
                row[vi[(p, c)]] = 1
        A_eq.append(row)
        b_eq.append(n[p])
    A_ub, b_ub = [], []
    for c in range(N_CORES):
        row = np.zeros(nv)
        for p in pairs:
            if (p, c) in vi:
                row[vi[(p, c)]] = 1
        A_ub.append(row)
        b_ub.append(NTOK)
        for e in blocks[c]:
            row = np.zeros(nv)
            for p in pairs:
                if e in p and (p, c) in vi:
                    row[vi[(p, c)]] = 1
            A_ub.append(row)
            b_ub.append((BIG_CAP if e in bigs[c] else SMALL_CAP) - margin)
    res = linprog(np.zeros(nv), A_ub=np.array(A_ub), b_ub=np.array(b_ub),
                  A_eq=np.array(A_eq), b_eq=np.array(b_eq),
                  bounds=[(0, None)] * nv, method='highs')
    if res.status != 0:
        return None
    x = res.x

    cores = np.full(top2.shape[0], -1, dtype=int)
    ecount = np.zeros((N_CORES, E), int)
    tcount = np.zeros(N_CORES, int)
    for p in pairs:
        if n[p] == 0:
            continue
        toks = np.where(pid == p[0] * E + p[1])[0]
        elig = [c for c in range(N_CORES) if (p, c) in vi]
        vals = np.array([x[vi[(p, c)]] for c in elig])
        ints = np.floor(vals).astype(int)
        rem = n[p] - ints.sum()
        frac = vals - ints
        for idx in np.argsort(-frac)[:rem]:
            ints[idx] += 1
        off = 0
        for c, k in zip(elig, ints.tolist()):
            cores[toks[off:off + k]] = c
            ecount[c, p[0]] += k
            ecount[c, p[1]] += k
            tcount[c] += k
            off += k
    for c in range(N_CORES):
        if tcount[c] > NTOK:
            return None
        for e in range(E):
            if e in blocks[c]:
                cap = BIG_CAP if e in bigs[c] else SMALL_CAP
                if ecount[c, e] > cap:
                    return None
            elif ecount[c, e] > 0:
                return None
    nbig = len(bigs[0])
    slot_caps = (4,) * nbig + (3,) * (5 - nbig)
    slot_experts = [sorted(bigs[c]) + sorted(set(blocks[c]) - bigs[c])
                    for c in range(N_CORES)]
    return slot_caps, slot_experts, cores


def _balance_tokens(top2):
    """Fallback: every core gets all 8 experts with shared per-expert caps."""
    g = np.bincount(top2.reshape(-1), minlength=E)
    cap_tiles = np.maximum(1, np.ceil(g / (128 * N_CORES)).astype(int))
    for _attempt in range(8):
        cap = cap_tiles * 128
        rem = np.tile(cap, (N_CORES, 1)).astype(int)
        ntok = np.zeros(N_CORES, dtype=int)
        cores = np.full(N_TOKENS, -1, dtype=int)
        slack = N_CORES * cap - g
        tok_score = np.minimum(slack[top2[:, 0]], slack[top2[:, 1]])
        order = np.argsort(tok_score, kind="stable")
        failed_expert = -1
        for t in order:
            e1, e2 = top2[t]
            room = np.minimum(rem[:, e1], rem[:, e2]).astype(float)
            room[ntok >= NTOK] = -1
            c = int(np.argmax(room + 1e-3 * rem.sum(axis=1)))
            if room[c] <= 0:
                failed_expert = e1 if rem[:, e1].max() <= 0 else e2
                break
            cores[t] = c
            rem[c, e1] -= 1
            rem[c, e2] -= 1
            ntok[c] += 1
        else:
            return tuple(int(c) for c in cap_tiles), cores
        cap_tiles[failed_expert] += 1
    raise RuntimeError("token balancing failed")


def _sigmoid(v):
    return 1.0 / (1.0 + np.exp(-v))


def _default_sched(T):
    """Combine m-tile m after `sched[m]` slot tiles have completed."""
    return tuple(min(T - (MT - 1) + m, T) for m in range(MT))


def _earliest_sched(T, maxtiles):
    """Tightest nondecreasing schedule: combine m fires once 128*(m+1)
    tokens (by sorted max slot tile) are complete, on every core."""
    sched = []
    prev = 1
    for m in range(MT):
        q = prev
        for mt in maxtiles:
            q = max(q, int(mt[128 * (m + 1) - 1]) + 1)
        sched.append(min(q, T))
        prev = sched[-1]
    sched[-1] = T
    return tuple(sched)


def _prepare_core(x, logits, top2, tok_ids, slot_experts, slot_caps):
    """Build one core's host arrays: gathered activations (xg), scatter
    indices (sc) and double-softmax slot weights (wsl). Returns
    (part, ordered_tok_ids, sorted_maxtile)."""
    nreal = len(tok_ids)
    assert nreal == NTOK, nreal
    t2 = top2[tok_ids]                              # [NTOK, 2]
    T = int(sum(slot_caps))
    S = T * 128
    NCH = (T + 3) // 4
    SP = NCH * GCH

    slot_tok = np.full(S, -1, dtype=np.int64)       # core-local token idx
    dvals = np.zeros(S, dtype=np.float32)           # v_self - v_other
    rank1 = np.zeros(S, dtype=bool)
    tile_of = np.zeros((NTOK, 2), dtype=int)        # [token, rank] -> tile
    off = 0
    for j, e in enumerate(slot_experts):
        sel = np.where((t2[:, 0] == e) | (t2[:, 1] == e))[0]
        assert len(sel) <= slot_caps[j] * 128, (j, e, len(sel))
        n = len(sel)
        sl = slice(off, off + n)
        slot_tok[sl] = sel
        other = np.where(t2[sel, 0] == e, t2[sel, 1], t2[sel, 0])
        gt = logits[tok_ids[sel]]
        dvals[sl] = gt[np.arange(n), e] - gt[np.arange(n), other]
        first = t2[sel, 0] == e
        rank1[off + np.arange(n)] = ~first
        tiles = off // 128 + np.arange(n) // 128
        tile_of[sel, np.where(first, 0, 1)] = tiles
        off += slot_caps[j] * 128

    # order core-local token ids by the last tile that feeds them, so
    # m-tile m's combine can fire as soon as its tiles are scattered
    maxtile = tile_of.max(axis=1)
    order = np.argsort(maxtile, kind="stable")
    newid = np.empty(NTOK, dtype=np.int64)
    newid[order] = np.arange(NTOK)

    real = slot_tok >= 0
    # double softmax over the top-2 pair, seen from this slot's expert
    wv = np.zeros(S, dtype=np.float32)
    wv[real] = _sigmoid(2.0 * _sigmoid(dvals[real]) - 1.0)
    lanes = np.arange(S) % 128
    sc = np.where(real,
                  2 * newid[np.where(real, slot_tok, 0)] + rank1,
                  2 * NTOK + lanes).astype(np.int32)

    # gathered slot activations, chunk-major & zero-padded:
    # XG[c, p, k, i] = x[tok(slot 512c+i), 128k+p]
    xs = np.zeros((SP, D), dtype=np.float32)
    xs[:S][real] = x[tok_ids[slot_tok[real]]]
    XG = np.ascontiguousarray(
        xs.reshape(NCH, GCH, KC, 128).transpose(0, 3, 2, 1)
    ).astype(ml_dtypes.bfloat16)

    part = {
        "xg": XG,                                                  # [NCH,128,KC,GCH]
        "sc": np.ascontiguousarray(
            sc.reshape(T, 128).T.astype(np.int32)),                # [128, T]
        "wsl": np.ascontiguousarray(
            wv.reshape(T, 128).T.astype(np.float32)),              # [128, T]
    }
    return part, tok_ids[order], maxtile[order]


def _pack_weights(We, be, slot_experts):
    idx = np.asarray(slot_experts, dtype=np.int64)
    NSLOT = len(idx)
    WSEG = np.ascontiguousarray(
        We[idx].reshape(NSLOT, KC, 128, OH, OHW).transpose(0, 3, 2, 1, 4)
    ).astype(ml_dtypes.bfloat16)
    bsel = be[idx].astype(np.float32)
    rows = np.stack([-bsel, np.exp(bsel)], axis=1).astype(ml_dtypes.bfloat16)
    BSEG = np.ascontiguousarray(
        np.broadcast_to(rows[:, :, None, :], (NSLOT, 2, 128, O)))
    return WSEG, BSEG


# ----------------------------------------------------------------------------
# Device program
# ----------------------------------------------------------------------------

def build_program(slot_caps, sched):
    slot_caps = tuple(int(c) for c in slot_caps)
    NSLOT = len(slot_caps)
    T = sum(slot_caps)
    NCH = (T + 3) // 4

    nc = bacc.Bacc("TRN2", target_bir_lowering=False, debug=False,
                   num_devices=N_CORES)

    xg = nc.dram_tensor("xg", [NCH, 128, KC, GCH], BF16,
                        kind="ExternalInput").ap()
    wseg = nc.dram_tensor("wseg", [NSLOT, OH, 128, KC, OHW], BF16,
                          kind="ExternalInput").ap()
    bseg = nc.dram_tensor("bseg", [NSLOT, 2, 128, O], BF16,
                          kind="ExternalInput").ap()
    scd = nc.dram_tensor("sc", [128, T], I32, kind="ExternalInput").ap()
    wsld = nc.dram_tensor("wsl", [128, T], F32, kind="ExternalInput").ap()
    out = nc.dram_tensor("out", [NTOK, O], BF16, kind="ExternalOutput").ap()
    # token-major rows: token i's rank0/rank1 rows adjacent at flat rows
    # 2i/2i+1; junk pad rows after 2*NTOK. The combine reads one [128,2*O]
    # block per m-tile; the scatter writes through the flat [.,O] view.
    rowsd = nc.dram_tensor("rowsd", [NTOK + 64, 2 * O], BF16).ap()
    rowsflat = rowsd.rearrange("n (two o) -> (n two) o", two=2)

    AF = mybir.ActivationFunctionType
    ALU = mybir.AluOpType

    with tile.TileContext(nc) as tc:
        with (
            tc.tile_pool(name="singles", bufs=1) as singles,
            tc.tile_pool(name="xgp", bufs=2) as xgp,
            tc.tile_pool(name="wpool", bufs=6) as wpool,
            tc.tile_pool(name="mpsum", bufs=6, space="PSUM") as mpsum,
            tc.tile_pool(name="wps", bufs=1, space="PSUM") as wpsp,
            tc.tile_pool(name="berp", bufs=2) as berp,
            tc.tile_pool(name="rowp", bufs=max(slot_caps) + 2) as rowp,
            tc.tile_pool(name="smallp", bufs=4) as smallp,
            tc.tile_pool(name="combp", bufs=4) as combp,
        ):
            # ---- PE warm-up: dependency-free matmuls flip the HAM clock
            # gate to 8/8 during the initial DMA wait.
            wz = singles.tile([128, 64], BF16)
            nc.vector.memset(wz, 0.0)
            wps = wpsp.tile([64, 64], F32)
            for i in range(NWARM):
                nc.tensor.matmul(wps, lhsT=wz, rhs=wz,
                                 start=(i % 8 == 0), stop=(i % 8 == 7))

            # ---- small shared inputs + zero-fill of out (scalar queue)
            sc_sb = singles.tile([128, T], I32)
            nc.scalar.dma_start(out=sc_sb, in_=scd)
            wsl_sb = singles.tile([128, T], F32)
            nc.scalar.dma_start(out=wsl_sb, in_=wsld)

            # ---- xg chunks + weight chunks interleaved on ONE queue (sync)
            # in consumption order; the first chunk/weight are split so the
            # first matmuls start as early as possible.
            xgc = [None] * NCH
            wsb = {}
            _off = 0
            for j in range(NSLOT):
                _t0, _t1 = _off, _off + slot_caps[j]
                _off = _t1
                newchunks = []
                for c in range(_t0 // 4, (_t1 - 1) // 4 + 1):
                    if xgc[c] is None:
                        xt = xgp.tile([128, KC, GCH], BF16, tag="xgc",
                                      name=f"xgc{c}")
                        if c > 0:
                            newchunks.append((xt, c))
                        xgc[c] = xt
                wtiles = []
                for oh in range(OH):
                    w = wpool.tile([128, KC, OHW], BF16, tag="wsb",
                                   name=f"w{j}_{oh}")
                    wsb[(j, oh)] = w
                    wtiles.append((w, wseg[j, oh]))
                if j == 0:
                    # interleave quarter-loads of the first chunk/weight so
                    # the first matmul groups start as early as possible
                    xt = xgc[0]
                    for q in range(4):
                        ksl = slice(4 * q, 4 * q + 4)
                        nc.sync.dma_start(out=xt[:, ksl, :], in_=xg[0, :, ksl, :])
                        nc.sync.dma_start(out=wtiles[0][0][:, ksl, :],
                                          in_=wtiles[0][1][:, ksl, :])
                    for w, src in wtiles[1:]:
                        nc.sync.dma_start(out=w, in_=src)
                else:
                    for w, src in wtiles:
                        nc.sync.dma_start(out=w, in_=src)
                for xt, c in newchunks:
                    nc.sync.dma_start(out=xt, in_=xg[c])

            pending = {}

            def combine_read(m, eng):
                # token p's two rows land adjacent: one 1MB contiguous read
                ab = combp.tile([128, 2 * O], BF16, tag="ab")
                eng.dma_start(out=ab, in_=rowsd[m * 128:(m + 1) * 128, :])
                pending[m] = ab

            def combine_add(m, eng):
                ab = pending.pop(m)
                nc.vector.tensor_tensor(out=ab[:, :O], in0=ab[:, :O],
                                        in1=ab[:, O:], op=ALU.add)
                eng.dma_start(out=out[m * 128:(m + 1) * 128, :],
                              in_=ab[:, :O])

            # ---- main loop: slot-major, oh-pass inside (weight chunks are
            # short-lived). exp(relu(h+be)) == exp(max(h,-be)) * exp(be):
            # the bias rides DVE ops with prebroadcast rows.
            negbe = {}
            expbe = {}

            def emit_ber(j):
                if j >= NSLOT or j in negbe:
                    return
                nb = berp.tile([128, O], BF16, tag="nrow", name=f"nrow{j}")
                nc.scalar.dma_start(out=nb, in_=bseg[j][0])
                eb = berp.tile([128, O], BF16, tag="erow", name=f"erow{j}")
                nc.scalar.dma_start(out=eb, in_=bseg[j][1])
                negbe[j] = nb
                expbe[j] = eb

            emit_ber(0)
            emit_ber(1)
            tiles_done = 0
            tile_off = 0
            for j in range(NSLOT):
                t0, t1 = tile_off, tile_off + slot_caps[j]
                tile_off = t1
                emit_ber(j + 1)
                rowbufs = {}
                sums = {}
                if j == 0:
                    touring = [(oh, t) for oh in range(OH)
                               for t in range(t0, t1)]
                else:
                    touring = [(oh, t) for g in (0, 2)
                               for t in range(t0, t1) for oh in (g, g + 1)]
                for oh, t in touring:
                        if oh == 0:
                            rowbufs[t] = rowp.tile([128, O], BF16, tag="rowbuf",
                                                   name=f"rowbuf{t}")
                            sums[t] = smallp.tile([128, OH], F32, tag="sums",
                                                  name=f"sums{t}")
                        ps = mpsum.tile([128, OHW], F32)
                        for k in range(KC):
                            nc.tensor.matmul(
                                ps,
                                lhsT=xgc[t // 4][:, k, (t % 4) * 128:
                                                 (t % 4) * 128 + 128],
                                rhs=wsb[(j, oh)][:, k, :],
                                start=(k == 0), stop=(k == KC - 1))
                        ohsl = slice(oh * OHW, (oh + 1) * OHW)
                        seg = rowbufs[t][:, ohsl]
                        nc.vector.tensor_tensor(out=seg, in0=ps,
                                                in1=negbe[j][:, ohsl],
                                                op=ALU.max)
                        nc.scalar.activation(seg, seg, AF.Exp)
                        nc.vector.tensor_tensor(out=seg, in0=seg,
                                                in1=expbe[j][:, ohsl],
                                                op=ALU.mult)
                        nc.vector.tensor_reduce(sums[t][:, oh:oh + 1], seg,
                                                axis=mybir.AxisListType.X,
                                                op=ALU.add)
                        if oh < OH - 1:
                            continue
                        # w/sum(exp) scale, then scatter-accumulate the
                        # tile's rows straight into the output tokens.
                        stot = smallp.tile([128, 1], F32, tag="stot")
                        nc.vector.tensor_reduce(stot, sums[t],
                                                axis=mybir.AxisListType.X,
                                                op=ALU.add)
                        nc.vector.reciprocal(stot, stot)
                        scl = smallp.tile([128, 1], F32, tag="scl")
                        nc.vector.tensor_tensor(out=scl, in0=stot,
                                                in1=wsl_sb[:, t:t + 1],
                                                op=ALU.mult)
                        nc.vector.tensor_scalar_mul(rowbufs[t], rowbufs[t],
                                                    scl[:, :1])
                        nc.gpsimd.indirect_dma_start(
                            out=rowsflat,
                            out_offset=bass.IndirectOffsetOnAxis(
                                ap=sc_sb[:, t:t + 1], axis=0),
                            in_=rowbufs[t], in_offset=None)
                        del rowbufs[t], sums[t]
                        tiles_done += 1
                        # mid-kernel combines: read deferred one tile (so its
                        # scatter-RAW wait is pre-satisfied on the scalar
                        # queue), add one tile later. End combines ride the
                        # (by now idle) sync queue at the earliest point.
                        for m in range(MT):
                            if sched[m] <= T - 2:
                                if sched[m] + 1 == tiles_done - 1:
                                    combine_add(m, nc.scalar)
                                elif sched[m] + 1 == tiles_done:
                                    combine_read(m, nc.scalar)
                            elif sched[m] == tiles_done:
                                combine_read(m, nc.sync)
            for m in sorted(pending):
                combine_add(m, nc.sync)

    nc.compile()
    return nc


_PROGRAM_CACHE = {}


def _get_program(key):
    if key not in _PROGRAM_CACHE:
        slot_caps, sched = key
        _PROGRAM_CACHE[key] = build_program(slot_caps, sched)
    return _PROGRAM_CACHE[key]


def make_in_maps(inputs, We, be, Wg, bg):
    """Returns (program_key, core_token_ids, in_maps)."""
    x = np.asarray(inputs, dtype=np.float32)
    We = np.asarray(We, dtype=np.float32)
    be = np.asarray(be, dtype=np.float32)
    Wg = np.asarray(Wg, dtype=np.float32)
    bg = np.asarray(bg, dtype=np.float32)

    logits, top2 = _host_route(x, Wg, bg)
    clus = _cluster_assign(top2)
    if clus is not None:
        slot_caps, slot_experts, cores = clus
    else:
        slot_caps, cores = _balance_tokens(top2)
        slot_experts = [list(range(E))] * N_CORES

    T = sum(slot_caps)
    parts, core_tok, maxtiles = [], [], []
    for c in range(N_CORES):
        tok = np.where(cores == c)[0]
        part, tok_ordered, mt = _prepare_core(
            x, logits, top2, tok, slot_experts[c], slot_caps)
        parts.append((part, slot_experts[c]))
        core_tok.append(tok_ordered)
        maxtiles.append(mt)
    sched = _earliest_sched(T, maxtiles)

    in_maps = []
    for c in range(N_CORES):
        part, sexp = parts[c]
        WSEG, BSEG = _pack_weights(We, be, sexp)
        part["wseg"] = WSEG
        part["bseg"] = BSEG
        in_maps.append(part)
    return (tuple(slot_caps), sched), core_tok, in_maps


def kernel(inputs, We, be, Wg, bg, top_x):
    assert int(top_x) == 2, "kernel specialized for top_x=2"
    key, core_tok, in_maps = make_in_maps(inputs, We, be, Wg, bg)
    nc = _get_program(key)
    res = run_bass_kernel_spmd(nc, in_maps, list(range(N_CORES)))
    full = np.empty((N_TOKENS, O), dtype=np.float32)
    for c in range(N_CORES):
        full[core_tok[c]] = np.asarray(res.results[c]["out"],
                                       dtype=np.float32)[:NTOK]
    return full
